# revision 28
# baseline (speedup 1.0000x reference)
"""L1 loss (mean |yhat - y|) over (64, 128, 4096) fp32 tensors on 8 TRN2 cores.

Strategy: pure data-parallel; core c takes 1/8 of the elements. The rel-err
budget (2e-2) is ~28x above fp8-e4m3 quantization error (7e-4 on the actual
inputs), so the host quantizes both tensors to fp8 and the kernel streams
2 bytes/element-pair instead of 8 — a 4x cut in HBM traffic.

Measured on HW, every DVE/ACT elementwise op runs ~1.2-1.3 ns/elem
regardless of dtype, so a sub + abs-reduce pipeline on those two engines is
compute-bound at ~44 us/core. All subtractions therefore run on the
otherwise-idle TENSOR engine: the host lays yhat on even SBUF partitions
and y on odd, and a [128 x 64] +/-1 stationary matrix turns each 512-column
matmul into 64x512 pairwise differences (fp8 in, exact fp32 out). Matmul
pairs fill the lo/hi 64-partition halves of [128 x 2048] PSUM tiles; DVE
(tensor_reduce, abs) takes the odd tiles and ACT (activation Abs,
accum_out) the even ones, writing fp32 columns of a [128, 16] accumulator
that the host sums in float64.

Synchronization is MANUAL (no TileContext) with counting semaphores, which
drops the tile framework's fixed preamble/epilogue sync ladder (~16 us).
Every wait rides ON a real instruction (1-wait-per-instruction HW limit):
 - s_dma (+16/completed DMA) gates the first matmul of each chunk.
 - s_mm (+1 per filled psum unit, PE program order) gates each reduce and
   gates chunk-buffer reuse on the input DMAs.
 - s_rd / s_ra (+1 per DVE / ACT reduce, each engine retires its units in
   order) gate psum buffer reuse; the WAR wait for unit i is attached to
   the LAST matmul of unit i-1, which precedes unit i's writes on the PE
   stream and has a free wait slot. The final DVE reduce (unit 15)
   increments s_ra instead of s_rd, so the output DMA's single wait
   (s_ra >= 9) transitively covers all 16 reduces.
"""

from contextlib import ExitStack

import numpy as np
import ml_dtypes

import concourse.bacc as bacc
import concourse.bass as bass
import concourse.mybir as mybir
from concourse.bass_utils import run_bass_kernel_spmd

N_CORES = 8
FULL_SHAPE = (64, 128, 4096)
TOTAL_ELEMS = FULL_SHAPE[0] * FULL_SHAPE[1] * FULL_SHAPE[2]  # 33,554,432

P = 128
PAIR_ROWS = 64                            # pairs per moving column
ELEMS_PER_CORE = TOTAL_ELEMS // N_CORES   # 4,194,304 pairs per core
N_COLS = ELEMS_PER_CORE // PAIR_ROWS      # 65,536 moving columns per core
MM_N = 512                                # moving cols per matmul (HW max)
PSUM_COLS = 2048                          # psum tile free size (4 banks)
COLS_PER_PSUM = 2 * PSUM_COLS             # 4096 moving cols per psum unit
N_UNITS = N_COLS // COLS_PER_PSUM         # 16
CHUNK = 8192                              # moving cols per input DMA (1 MiB)
N_CHUNKS = N_COLS // CHUNK                # 8
N_IOBUF = 3                               # SBUF chunk buffers in flight
UNITS_PER_CHUNK = CHUNK // COLS_PER_PSUM  # 2
MM_PER_UNIT = 2 * (PSUM_COLS // MM_N)     # 8

IN_DT = mybir.dt.float8e4
IN_NP = ml_dtypes.float8_e4m3

_nc_cache = []


def _build_nc():
    nc = bacc.Bacc("TRN2", target_bir_lowering=False, debug=False)
    z = nc.declare_dram_parameter("z", [P, N_COLS], IN_DT, isOutput=False)
    w = nc.declare_dram_parameter("w", [P, PAIR_ROWS], IN_DT, isOutput=False)
    out = nc.declare_dram_parameter("out", [P, N_UNITS], mybir.dt.float32, isOutput=True)

    s_dma = nc.alloc_semaphore("s_dma")   # +16 per completed input DMA
    s_mm = nc.alloc_semaphore("s_mm")     # +1 per psum unit filled
    s_rd = nc.alloc_semaphore("s_rd")     # +1 per DVE reduce (odd units 1..13)
    s_ra = nc.alloc_semaphore("s_ra")     # +1 per ACT reduce + the final DVE one
    s_aux = nc.alloc_semaphore("s_aux")   # scrap: walrus codegen requires an
                                          # update on every wait-carrying inst

    with ExitStack() as ctx:
        wt = ctx.enter_context(nc.sbuf_tensor("wt", [P, PAIR_ROWS], IN_DT))
        bufs = [
            ctx.enter_context(nc.sbuf_tensor(f"zb{k}", [P, CHUNK], IN_DT))
            for k in range(N_IOBUF)
        ]
        scr = ctx.enter_context(
            nc.sbuf_tensor("scr", [P, PSUM_COLS], mybir.dt.bfloat16)
        )
        acc = ctx.enter_context(nc.sbuf_tensor("acc", [P, N_UNITS], mybir.dt.float32))
        pts = [
            ctx.enter_context(nc.psum_tensor(f"pt{k}", [P, PSUM_COLS], mybir.dt.float32))
            for k in range(2)
        ]

        # SP: weights DMA, then input chunks (throttled by PE buffer reuse).
        nc.sync.dma_start(wt.ap(), w.ap()).then_inc(s_dma, 16)
        for c in range(N_CHUNKS):
            dma = nc.sync.dma_start(
                bufs[c % N_IOBUF].ap(), z.ap()[:, c * CHUNK : (c + 1) * CHUNK]
            ).then_inc(s_dma, 16)
            if c >= N_IOBUF:
                # Buffer reuse: chunk c-N_IOBUF fully consumed by PE.
                dma._wait_ge(s_mm, UNITS_PER_CHUNK * (c - N_IOBUF + 1))

        # PE: 16 matmuls per chunk -> 2 psum units; DVE/ACT: abs+sum reduces.
        for c in range(N_CHUNKS):
            zt = bufs[c % N_IOBUF].ap()
            for u in range(UNITS_PER_CHUNK):
                i = c * UNITS_PER_CHUNK + u
                pt = pts[i % 2].ap()
                last = None
                for k in range(MM_PER_UNIT):
                    half = k % 2
                    qc = (k // 2) * MM_N
                    s = u * MM_PER_UNIT + k
                    last = nc.tensor.matmul(
                        pt[half * PAIR_ROWS : (half + 1) * PAIR_ROWS, qc : qc + MM_N],
                        wt.ap(),
                        zt[:, s * MM_N : (s + 1) * MM_N],
                        start=True,
                        stop=True,
                    )
                    if k == 0 and u == 0:
                        # Chunk data (and the weights) have landed.
                        last._wait_ge(s_dma, 16 * (c + 2))
                        last.then_inc(s_aux)
                last.then_inc(s_mm)
                if i + 1 < N_UNITS:
                    # WAR for unit i+1's psum buffer, attached here: this
                    # instruction precedes unit i+1's first write on the PE
                    # stream and has a free wait slot.
                    j = i + 1
                    if j >= 2:
                        if j % 2 == 1:
                            last._wait_ge(s_rd, (j - 1) // 2)
                        else:
                            last._wait_ge(s_ra, j // 2)
                # Reduce this unit on its engine.
                if i % 2 == 1:
                    red_sem = s_ra if i == N_UNITS - 1 else s_rd
                    nc.vector.tensor_reduce(
                        acc.ap()[:, i : i + 1], pt,
                        axis=mybir.AxisListType.X, op=mybir.AluOpType.add,
                        apply_absolute_value=True,
                    ).then_inc(red_sem)._wait_ge(s_mm, i + 1)
                else:
                    nc.scalar.activation(
                        scr.ap(), pt, mybir.ActivationFunctionType.Abs,
                        accum_out=acc.ap()[:, i : i + 1],
                    ).then_inc(s_ra)._wait_ge(s_mm, i + 1)

        # All 16 reduces done: ACT's 8 + DVE's last imply DVE's earlier 7.
        nc.sync.dma_start(out.ap(), acc.ap())._wait_ge(
            s_ra, N_UNITS // 2 + 1
        ).then_inc(s_aux, 16)
    nc.compile()
    return nc


def _get_nc():
    if not _nc_cache:
        _nc_cache.append(_build_nc())
    return _nc_cache[0]


def _shard_inputs(yhat: np.ndarray, y: np.ndarray) -> list[dict[str, np.ndarray]]:
    yhat8 = np.ascontiguousarray(yhat, dtype=np.float32).astype(IN_NP)
    y8 = np.ascontiguousarray(y, dtype=np.float32).astype(IN_NP)
    # Core c: pairs laid out [64 pair-rows, N_COLS]; yhat on even partitions,
    # y on odd.
    a = yhat8.reshape(N_CORES, PAIR_ROWS, N_COLS)
    b = y8.reshape(N_CORES, PAIR_ROWS, N_COLS)
    z = np.empty((N_CORES, PAIR_ROWS, 2, N_COLS), dtype=IN_NP)
    z[:, :, 0, :] = a
    z[:, :, 1, :] = b
    z = z.reshape(N_CORES, P, N_COLS)
    # +/-1 pair-difference weights: out[k] = z[2k] - z[2k+1]
    w = np.zeros((P, PAIR_ROWS), dtype=IN_NP)
    for k in range(PAIR_ROWS):
        w[2 * k, k] = 1.0
        w[2 * k + 1, k] = -1.0
    return [{"z": z[c], "w": w} for c in range(N_CORES)]


def kernel(yhat: np.ndarray, y: np.ndarray) -> np.ndarray:
    nc = _get_nc()
    in_maps = _shard_inputs(yhat, y)
    res = run_bass_kernel_spmd(nc, in_maps, list(range(N_CORES)))
    total = np.float64(0.0)
    for r in res.results:
        total += r["out"].astype(np.float64).sum()
    return np.asarray(total / TOTAL_ELEMS, dtype=np.float32)


# revision 29
# speedup vs baseline: 1.3805x; 1.3805x over previous
"""L1 loss (mean |yhat - y|) over (64, 128, 4096) fp32 tensors on 8 TRN2 cores.

Strategy: pure data-parallel; core c takes 1/8 of the elements. The rel-err
budget (2e-2) is ~28x above fp8-e4m3 quantization error (7e-4 on the actual
inputs), so the host quantizes both tensors to fp8 and the kernel streams
2 bytes/element-pair instead of 8 — a 4x cut in HBM traffic.

Measured on HW, every DVE/ACT elementwise op runs ~1.2-1.3 ns/elem
regardless of dtype (no fast modes engage), so a sub + abs-reduce pipeline
on those two engines alone is compute-bound at ~44 us/core — well above the
~27 us fp8 DMA floor. This kernel instead computes ALL subtractions on the
otherwise-idle TENSOR engine: the host lays yhat on even SBUF partitions
and y on odd, and a [128 x 64] +/-1 stationary matrix turns each 512-column
matmul into 64x512 pairwise differences (fp8 in, exact fp32 out). Matmul
pairs fill the lo/hi 64-partition halves of [128 x 2048] PSUM tiles
(4 banks each, 2 in flight = all 8). DVE (tensor_reduce, abs) and ACT
(activation Abs, accum_out) split the 16 per-core abs+sum reductions, each
~2.3 us per tile, writing fp32 columns of a [128, 18] accumulator. The last
two PSUM tiles are reduced as two 1024-col halves on DVE+ACT concurrently
to shorten the tail. Input DMAs are issued weights-first then
0.25->2 MiB chunks so the PE starts ~2 us earlier; DMA supply (~300 B/ns)
and PE (~273 B/ns effective) are the co-poles. Host sums in float64.
"""

import numpy as np
import ml_dtypes

import concourse.bacc as bacc
import concourse.bass as bass
import concourse.mybir as mybir
import concourse.tile as tile
from concourse.bass_utils import run_bass_kernel_spmd

N_CORES = 8
FULL_SHAPE = (64, 128, 4096)
TOTAL_ELEMS = FULL_SHAPE[0] * FULL_SHAPE[1] * FULL_SHAPE[2]  # 33,554,432

P = 128
PAIR_ROWS = 64                            # pairs per moving column
ELEMS_PER_CORE = TOTAL_ELEMS // N_CORES   # 4,194,304 pairs per core
N_COLS = ELEMS_PER_CORE // PAIR_ROWS      # 65,536 moving columns per core
MM_N = 512                                # moving cols per matmul (HW max)
PSUM_COLS = 1024                          # psum tile free size (2 banks)
COLS_PER_PSUM = 2 * PSUM_COLS             # 4096 moving cols -> one psum tile
N_UNITS = N_COLS // COLS_PER_PSUM         # 16
DMA_CHUNKS = [2048, 6144, 8192, 16384, 16384, 16384]  # 0.25->2 MiB
assert sum(DMA_CHUNKS) == N_COLS
N_ACC = N_UNITS + 2                       # last two units split into halves

IN_DT = mybir.dt.float8e4
IN_NP = ml_dtypes.float8_e4m3

_nc_cache = []


def _build_nc():
    nc = bacc.Bacc("TRN2", target_bir_lowering=False, debug=False)
    z = nc.declare_dram_parameter("z", [P, N_COLS], IN_DT, isOutput=False)
    w = nc.declare_dram_parameter("w", [P, PAIR_ROWS], IN_DT, isOutput=False)
    out = nc.declare_dram_parameter("out", [P, N_ACC], mybir.dt.float32, isOutput=True)

    with tile.TileContext(nc) as tc:
        with (
            tc.tile_pool(name="io", bufs=3) as io_pool,
            tc.tile_pool(name="wp", bufs=1) as w_pool,
            tc.tile_pool(name="ps", bufs=4, space="PSUM") as psum_pool,
            tc.tile_pool(name="scr", bufs=2) as scr_pool,
            tc.tile_pool(name="acc", bufs=1) as acc_pool,
        ):
            wt = w_pool.tile([P, PAIR_ROWS], IN_DT)
            nc.sync.dma_start(wt[:], w[:, :])
            acc = acc_pool.tile([P, N_ACC], mybir.dt.float32)

            col = 0
            psum_idx = 0
            pt = None
            pt_fill = 0
            for chunk in DMA_CHUNKS:
                zt = io_pool.tile([P, chunk], IN_DT, tag="z")
                nc.sync.dma_start(zt[:], z[:, col : col + chunk])
                col += chunk
                for s in range(chunk // MM_N):
                    if pt is None:
                        pt = psum_pool.tile([P, PSUM_COLS], mybir.dt.float32, tag="ps")
                        pt_fill = 0
                    half = pt_fill % 2
                    qc = (pt_fill // 2) * MM_N
                    nc.tensor.matmul(
                        pt[half * PAIR_ROWS : (half + 1) * PAIR_ROWS, qc : qc + MM_N],
                        wt[:],
                        zt[:, s * MM_N : (s + 1) * MM_N],
                        start=True,
                        stop=True,
                    )
                    pt_fill += 1
                    if pt_fill == 2 * (PSUM_COLS // MM_N):
                        i = psum_idx
                        if i >= N_UNITS - 2:
                            # Tail: reduce the two 1024-col halves on DVE and
                            # ACT concurrently.
                            h2 = PSUM_COLS // 2
                            nc.vector.tensor_reduce(
                                acc[:, i : i + 1], pt[:, 0:h2],
                                axis=mybir.AxisListType.X, op=mybir.AluOpType.add,
                                apply_absolute_value=True,
                            )
                            scr = scr_pool.tile([P, h2], mybir.dt.bfloat16, tag="sh")
                            ac = N_UNITS + (i - (N_UNITS - 2))
                            nc.scalar.activation(
                                scr[:], pt[:, h2:PSUM_COLS],
                                mybir.ActivationFunctionType.Abs,
                                accum_out=acc[:, ac : ac + 1],
                            )
                        elif i % 2 == 1:
                            nc.vector.tensor_reduce(
                                acc[:, i : i + 1], pt[:],
                                axis=mybir.AxisListType.X, op=mybir.AluOpType.add,
                                apply_absolute_value=True,
                            )
                        else:
                            scr = scr_pool.tile(
                                [P, PSUM_COLS], mybir.dt.bfloat16, tag="sa"
                            )
                            nc.scalar.activation(
                                scr[:], pt[:], mybir.ActivationFunctionType.Abs,
                                accum_out=acc[:, i : i + 1],
                            )
                        psum_idx += 1
                        pt = None
            assert pt is None and psum_idx == N_UNITS
            nc.sync.dma_start(out[:], acc[:])
    nc.compile()
    return nc


def _get_nc():
    if not _nc_cache:
        _nc_cache.append(_build_nc())
    return _nc_cache[0]


def _shard_inputs(yhat: np.ndarray, y: np.ndarray) -> list[dict[str, np.ndarray]]:
    yhat8 = np.ascontiguousarray(yhat, dtype=np.float32).astype(IN_NP)
    y8 = np.ascontiguousarray(y, dtype=np.float32).astype(IN_NP)
    # Core c: pairs laid out [64 pair-rows, N_COLS]; yhat on even partitions,
    # y on odd.
    a = yhat8.reshape(N_CORES, PAIR_ROWS, N_COLS)
    b = y8.reshape(N_CORES, PAIR_ROWS, N_COLS)
    z = np.empty((N_CORES, PAIR_ROWS, 2, N_COLS), dtype=IN_NP)
    z[:, :, 0, :] = a
    z[:, :, 1, :] = b
    z = z.reshape(N_CORES, P, N_COLS)
    # +/-1 pair-difference weights: out[k] = z[2k] - z[2k+1]
    w = np.zeros((P, PAIR_ROWS), dtype=IN_NP)
    for k in range(PAIR_ROWS):
        w[2 * k, k] = 1.0
        w[2 * k + 1, k] = -1.0
    return [{"z": z[c], "w": w} for c in range(N_CORES)]


def kernel(yhat: np.ndarray, y: np.ndarray) -> np.ndarray:
    nc = _get_nc()
    in_maps = _shard_inputs(yhat, y)
    res = run_bass_kernel_spmd(nc, in_maps, list(range(N_CORES)))
    total = np.float64(0.0)
    for r in res.results:
        total += r["out"].astype(np.float64).sum()
    return np.asarray(total / TOTAL_ELEMS, dtype=np.float32)
